# revision 52
# baseline (speedup 1.0000x reference)
"""Multi-head attention TRN2 kernel (8 NeuronCores), v3.

Sharding: data parallel on batch (B=2, 4 cores each), tensor parallel on
heads (4 of 16 heads per core; wq/wk/wv column-parallel, wo row-parallel).
Each core computes a partial [D, S] transposed output (bf16) for its batch;
the host upcasts, sums the 4 partials per batch, transposes, and adds bo.

v3 vs v2:
- attn@V head pairs run CONCURRENTLY via PE column tiling (M=64 each,
  tile_position (0,0)/(0,64) auto-derived from psum base partitions).  The
  softmax denominator no longer rides the attn@V matmul as a 65th weight
  column; instead the exp tiles are accumulated on the DVE (bf16 et_sum per
  group) and one column-tiled ones-matmul pair reduces+broadcasts the
  denominators at group end.  This also aligns the po psum layout with
  ot_sb rows 0..127, removing the per-group partition-shift DMA.
- exp ops for causally-restricted slots use one 3D-AP ACTIVATE ([128,2,n])
  instead of two, saving the 352-cycle per-op ACT overhead.
- ramp: warmup matmuls bridge the framework preamble to the first DMAs,
  weights+stage0 inputs are split per-chunk across all three DMA queues so
  the first projections trickle-start ~15us earlier; stage1-3 inputs move
  as per-chunk [128,3*512] row-tails (3KB runs) to cut DMA-issue cost.
- tail: the last output-projection units rotate through 6 psum banks, cast
  on alternating DVE/GpSimd, and spread store-DMAs over all 3 queues.
"""

import numpy as np
import ml_dtypes

import concourse.bass as bass
import concourse.mybir as mybir
import concourse.tile as tile
from concourse import bacc, bass_isa
from concourse.bass_utils import run_bass_kernel_spmd

B = 2
S = 2048
D_MODEL = 1024
NUM_HEADS = 16
DEPTH = 64
NEG = -1e9
N_CORES = 8
CORES_PER_BATCH = 4
HEADS_PER_CORE = 4           # 2 head-pairs (bi) x 2 (hp)
DC = HEADS_PER_CORE * DEPTH  # 256
QT = 512                     # query tile
KB = 128                     # key block (one slot per valid (bi, t, kb))
NQT = S // QT                # 4
NKB = S // KB                # 16
KIN = D_MODEL // 128         # 8 contraction chunks

F32 = mybir.dt.float32
BF16 = mybir.dt.bfloat16
EXP = mybir.ActivationFunctionType.Exp

_cache = {}


def _build(slot_plan, n_full_masks):
    """slot_plan[(t, kb)] = None (skip) | (q0, kind, idx)
    kind: 0 plain, 1 tri (leading 128-triangle at q0), 2 full mask tile idx.
    """
    nc = bacc.Bacc("TRN2", target_bir_lowering=False, debug=False,
                   num_devices=N_CORES)

    xqT = nc.dram_tensor("xqT", [D_MODEL, S], BF16, kind="ExternalInput").ap()
    xkT = nc.dram_tensor("xkT", [D_MODEL, S], BF16, kind="ExternalInput").ap()
    xvT = nc.dram_tensor("xvT", [D_MODEL, S], BF16, kind="ExternalInput").ap()
    wq = nc.dram_tensor("wq", [D_MODEL, DC], BF16, kind="ExternalInput").ap()
    wk = nc.dram_tensor("wk", [D_MODEL, DC], BF16, kind="ExternalInput").ap()
    wv = nc.dram_tensor("wv", [D_MODEL, DC], BF16, kind="ExternalInput").ap()
    wo = nc.dram_tensor("wo", [DC, D_MODEL], BF16, kind="ExternalInput").ap()
    bq = nc.dram_tensor("bq", [128, 2], F32, kind="ExternalInput").ap()
    bk = nc.dram_tensor("bk", [128, 2], F32, kind="ExternalInput").ap()
    bv = nc.dram_tensor("bv", [128, DC], F32, kind="ExternalInput").ap()
    tri = nc.dram_tensor("tri", [KB, 2 * KB], BF16,
                         kind="ExternalInput").ap()
    ident = nc.dram_tensor("ident", [KB, KB], BF16,
                           kind="ExternalInput").ap()
    fmask = nc.dram_tensor("fmask", [max(n_full_masks, 1), KB, QT], F32,
                           kind="ExternalInput").ap()
    outT = nc.dram_tensor("outT", [D_MODEL, S], BF16,
                          kind="ExternalOutput").ap()

    with tile.TileContext(nc) as tc:
        import contextlib
        ctx = contextlib.ExitStack()
        with ctx:
            wpool = ctx.enter_context(tc.tile_pool(name="weights", bufs=1))
            xpool = ctx.enter_context(tc.tile_pool(name="xin", bufs=1))
            qkv = ctx.enter_context(tc.tile_pool(name="qkv", bufs=1))
            etp = ctx.enter_context(tc.tile_pool(name="etp", bufs=6))
            nrm = ctx.enter_context(tc.tile_pool(name="nrm", bufs=2))
            ostp = ctx.enter_context(tc.tile_pool(name="ostp", bufs=4))
            psL = ctx.enter_context(
                tc.tile_pool(name="psL", bufs=2, space="PSUM"))
            psO = ctx.enter_context(
                tc.tile_pool(name="psO", bufs=2, space="PSUM"))
            psF = ctx.enter_context(
                tc.tile_pool(name="psF", bufs=2, space="PSUM"))
            # PSUM banks: psL 2x2 + psO 2x1 + psF 2x1 = 8

            # ---- resident weights / constants ----------------------------
            wq_sb = wpool.tile([128, KIN, DC], BF16, tag="wq")
            wk_sb = wpool.tile([128, KIN, DC], BF16, tag="wk")
            wv_sb = wpool.tile([128, KIN, DC], BF16, tag="wv")
            wo_sb = wpool.tile([128, 2, D_MODEL], BF16, tag="wo")
            bq_sb = wpool.tile([128, 2], F32, tag="bq")
            bk_sb = wpool.tile([128, 2], F32, tag="bk")
            bv_sb = wpool.tile([128, DC], F32, tag="bv")
            tri_sb = wpool.tile([KB, 2 * KB], BF16, tag="tri")
            ident_sb = wpool.tile([KB, KB], BF16, tag="ident")
            fm_sb = []
            for i in range(n_full_masks):
                fm = wpool.tile([KB, QT], F32, tag=f"fm{i}", name=f"fm{i}")
                nc.gpsimd.dma_start(fm[:], fmask[i])
                fm_sb.append(fm)

            dummy = wpool.tile([128, QT], BF16, tag="dummy")
            nc.vector.memset(dummy[:], 0.25)
            # [128, 64] of ones: stationary for the denominator
            # reduce+broadcast matmuls (col-tiled pair per group)
            ones64 = wpool.tile([128, 64], BF16, tag="ones64")
            nc.vector.memset(ones64[:], 1.0)

            # persistent activations
            qt_sb = [qkv.tile([128, S], BF16, tag=f"qt{i}", name=f"qt{i}")
                     for i in range(2)]
            kt_sb = [qkv.tile([128, S], BF16, tag=f"kt{i}", name=f"kt{i}")
                     for i in range(2)]
            ot_sb = [qkv.tile([128, S], BF16, tag=f"ot{i}", name=f"ot{i}")
                     for i in range(2)]
            v_sb = qkv.tile([128, NKB, HEADS_PER_CORE, DEPTH], BF16,
                            tag="v")

            # ---- DMA issue ------------------------------------------------
            # warm the exp table ASAP (gated only on dummy memset)
            et_warm = etp.tile([128, 2, QT], BF16, tag="et", name="etwarm")
            nc.scalar.activation(et_warm[:, 0, 0:8], dummy[:, 0:8], EXP)

            # x resident
            xq_sb = xpool.tile([128, KIN, S], BF16, tag="xq")
            xk_sb = xpool.tile([128, KIN, S], BF16, tag="xk")
            xv_sb = xpool.tile([128, KIN, S], BF16, tag="xv")

            # Each HWDGE ring serializes transfers at a ~2us fixed latency
            # apiece, so the ramp-critical loads go as ONE batched transfer
            # per (tensor, stage): q-side on sync, k-side on gpsimd, v-side
            # stage 0 on scalar.  Issue order on a queue is its priority.
            def xst(q, dst, srct, st):
                sl = slice(st * QT, (st + 1) * QT)
                q.dma_start(dst[:, :, sl],
                            srct[:, sl].rearrange("(c p) q -> p c q", p=128))

            # Measured queue speeds: gpsimd (SWDGE) ~210GB/s, scalar ~80,
            # sync ~60.  The bulk flows through gpsimd in need order; the
            # two slow HWDGE rings carry small slices off the critical path.
            def xst0(q, dst, srct, c0, c1):
                q.dma_start(
                    dst[:, c0:c1, 0:QT],
                    srct[c0 * 128:c1 * 128, 0:QT].rearrange(
                        "(c p) q -> p c q", p=128))

            # ramp is HBM-bound across the 8 cores: ship the L0-critical
            # bytes first, per-chunk so compute trickle-starts.
            nc.sync.dma_start(bq_sb[:], bq[:])
            nc.sync.dma_start(
                wq_sb[:], wq.rearrange("(c p) d -> p c d", p=128))
            for ch in range(6):
                nc.sync.dma_start(xq_sb[:, ch, 0:QT],
                                  xqT[ch * 128:(ch + 1) * 128, 0:QT])
            nc.gpsimd.dma_start(bk_sb[:], bk[:])
            nc.gpsimd.dma_start(bv_sb[:], bv[:])
            nc.gpsimd.dma_start(tri_sb[:], tri[:])
            nc.gpsimd.dma_start(ident_sb[:], ident[:])
            nc.gpsimd.dma_start(
                wk_sb[:], wk.rearrange("(c p) d -> p c d", p=128))
            for ch in (6, 7):
                nc.gpsimd.dma_start(xq_sb[:, ch, 0:QT],
                                    xqT[ch * 128:(ch + 1) * 128, 0:QT])
            for ch in range(KIN):
                nc.gpsimd.dma_start(xk_sb[:, ch, 0:QT],
                                    xkT[ch * 128:(ch + 1) * 128, 0:QT])
            # v-side stage 0 spread per-chunk across all three queues so the
            # early attn@V units trickle with the landings
            nc.scalar.dma_start(
                wv_sb[:], wv.rearrange("(c p) d -> p c d", p=128))
            for ch in (6, 7):
                nc.scalar.dma_start(xv_sb[:, ch, 0:QT],
                                    xvT[ch * 128:(ch + 1) * 128, 0:QT])
            for ch in (0, 1, 2):
                nc.sync.dma_start(xv_sb[:, ch, 0:QT],
                                  xvT[ch * 128:(ch + 1) * 128, 0:QT])
            for ch in (3, 4, 5):
                nc.gpsimd.dma_start(xv_sb[:, ch, 0:QT],
                                    xvT[ch * 128:(ch + 1) * 128, 0:QT])
            # stages 1-3 as stage-batched ~1MB transfers, issued in need
            # order: xq/xv on sync, xk on gpsimd
            # all q/k stage batches ride the fast gpsimd ring (they gate the
            # group boundaries); v-side stages go on sync
            xst(nc.gpsimd, xq_sb, xqT, 1)
            xst(nc.gpsimd, xk_sb, xkT, 1)
            nc.gpsimd.dma_start(
                wo_sb[:], wo.rearrange("(c p) d -> p c d", p=128))
            xst(nc.sync, xv_sb, xvT, 1)
            xst(nc.gpsimd, xq_sb, xqT, 2)
            xst(nc.gpsimd, xk_sb, xkT, 2)
            xst(nc.sync, xv_sb, xvT, 2)
            xst(nc.gpsimd, xq_sb, xqT, 3)
            xst(nc.gpsimd, xk_sb, xkT, 3)
            xst(nc.sync, xv_sb, xvT, 3)

            # ---- PE warmup (HAM) while DMAs land --------------------------
            # a short dummy block, then one matmul per landing xq stage-0
            # chunk.  NOTE: more/denser warmup consistently regresses the
            # whole kernel ~15-20% (power-driven clock throttling) -- keep
            # this minimal.
            for i in range(12):
                pw = psF.tile([128, QT], F32, tag="f", name=f"warm{i}")
                nc.tensor.matmul(pw[:, 0:256], dummy[:, 0:128],
                                 dummy[:, 0:256], start=True, stop=True)
            for ch in range(KIN):
                pw = psF.tile([128, QT], F32, tag="f", name=f"warmx{ch}")
                nc.tensor.matmul(pw[:, 0:256], dummy[:, 0:128],
                                 xq_sb[:, ch, 0:256], start=True, stop=True)

            # ---- filler units --------------------------------------------
            def emit_qk_unit(which, m, st):
                wsb = wq_sb if which == "q" else wk_sb
                bsb = bq_sb if which == "q" else bk_sb
                dst = (qt_sb if which == "q" else kt_sb)[m]
                ps = [None]
                sl = slice(st * QT, (st + 1) * QT)

                xsb = xq_sb if which == "q" else xk_sb

                def mk(ch):
                    def go():
                        if ch == 0:
                            ps[0] = psF.tile([128, QT], F32, tag="f",
                                             name=f"p{which}{m}{st}")
                        nc.tensor.matmul(
                            ps[0][:], wsb[:, ch, m * 128:(m + 1) * 128],
                            xsb[:, ch, sl],
                            start=(ch == 0), stop=(ch == KIN - 1))
                        return 512
                    return go

                steps = [mk(ch) for ch in range(KIN)]

                def fin():
                    nc.vector.tensor_scalar_add(dst[:, sl], ps[0][:],
                                                bsb[:, m:m + 1])
                    return 0
                steps.append(fin)
                return steps

            def emit_v_unit(si):
                ps = [None]

                def mk(ch):
                    def go():
                        if ch == 0:
                            ps[0] = psF.tile([128, QT], F32, tag="f",
                                             name=f"pv{si}")
                        nc.tensor.matmul(
                            ps[0][:, 0:DC],
                            xv_sb[:, ch, si * 128:(si + 1) * 128],
                            wv_sb[:, ch, :],
                            start=(ch == 0), stop=(ch == KIN - 1))
                        return 256
                    return go

                steps = [mk(ch) for ch in range(KIN)]

                def fin():
                    nc.vector.tensor_add(
                        v_sb[:, si, :, :],
                        ps[0][:, 0:DC].rearrange("p (h d) -> p h d",
                                                 h=HEADS_PER_CORE),
                        bv_sb[:].rearrange("p (h d) -> p h d",
                                           h=HEADS_PER_CORE))
                    return 0
                steps.append(fin)
                return steps

            def emit_o_unit(dt, sq, ps_ap=None, ceng=None, dq=None):
                ps = [ps_ap]
                sl = slice(sq * QT, (sq + 1) * QT)

                def mk(bi):
                    def go():
                        if bi == 0 and ps[0] is None:
                            ps[0] = psF.tile([128, QT], F32, tag="f",
                                             name=f"po{dt}{sq}")[:]
                        nc.tensor.matmul(
                            ps[0], wo_sb[:, bi, dt * 128:(dt + 1) * 128],
                            ot_sb[bi][:, sl],
                            start=(bi == 0), stop=(bi == 1))
                        return 512
                    return go

                steps = [mk(bi) for bi in range(2)]

                def fin():
                    ost = ostp.tile([128, QT], BF16, tag="ost",
                                    name=f"os{dt}{sq}")
                    # alternate DVE / ACT for the psum->bf16 cast: DVE is
                    # the busier engine mid-kernel, ACT has idle gaps
                    ce = ceng if ceng is not None else \
                        (nc.vector if dt % 2 == 0 else nc.scalar)
                    if ce is nc.scalar:
                        ce.copy(ost[:], ps[0])
                    else:
                        ce.tensor_copy(ost[:], ps[0])
                    q = dq if dq is not None else \
                        (nc.sync if (dt + sq) % 2 == 0 else nc.gpsimd)
                    q.dma_start(outT[dt * 128:(dt + 1) * 128, sl], ost[:])
                    return 0
                steps.append(fin)
                return steps

            filler = []    # forced entries: (need_key, est, step_fn)
            filler_o = []  # opportunistic o-units: (sq, est, step_fn)
            normed = set()  # groups whose norm has been emitted

            def pump(cycles):
                took = 0
                while filler and took < cycles:
                    took += filler.pop(0)[2]()
                # o-units only once both source groups' norms are emitted
                while filler_o and took < cycles:
                    sq = filler_o[0][0]
                    if 2 * sq in normed and 2 * sq + 1 in normed:
                        took += filler_o.pop(0)[2]()
                    else:
                        break
                return took

            def drain_key(key):
                # scan the whole list: need keys are not globally sorted
                i = 0
                while i < len(filler):
                    if filler[i][0] <= key:
                        filler.pop(i)[2]()
                    else:
                        i += 1

            # ---- attention machinery -------------------------------------
            groups = []
            for t in range(NQT):
                for bi in range(2):
                    slots = []
                    for kb in range(NKB):
                        d = slot_plan.get((t, kb))
                        if d is not None:
                            slots.append((kb,) + d)
                    groups.append((bi, t, slots))

            state = {"pend": [], "po": {}, "es": {}, "done_av": {},
                     "n_in_group": {}}

            def emit_L(bi, t, kb, kind, tag):
                # logits for both head-pairs, row-tiled (K=64 strips at
                # partitions 0/64 -> the two matmuls run concurrently)
                lg = psL.tile([128, 2, QT], F32, tag="lg", name=f"lg{tag}")
                qs = slice(t * QT, (t + 1) * QT)
                for hp in range(2):
                    prow = slice(hp * 64, hp * 64 + 64)
                    nc.tensor.matmul(
                        lg[:, hp, :],
                        kt_sb[bi][prow, kb * KB:(kb + 1) * KB],
                        qt_sb[bi][prow, qs], start=True,
                        stop=(kind != 1))
                return lg

            def emit_fin(g, si, q0, kind, idx, lg, tag):
                if kind == 1:
                    # accumulate the causal triangle on the PE (ident @ tri)
                    for hp in range(2):
                        nc.tensor.matmul(lg[:, hp, q0:q0 + KB], ident_sb[:],
                                         tri_sb[:, 0:KB],
                                         start=False, stop=True)
                elif kind == 2:
                    for hp in range(2):
                        nc.vector.tensor_add(lg[:, hp, :], lg[:, hp, :],
                                             fm_sb[idx][:])
                et = etp.tile([128, 2, QT], BF16, tag="et", name=f"et{tag}")
                full = (q0 == 0 or kind == 2)
                if full:
                    nc.scalar.activation(et[:], lg[:], EXP)
                else:
                    # one 3D-AP op covers both head-pairs' valid columns
                    nc.scalar.activation(et[:, :, q0:], lg[:, :, q0:], EXP)
                # accumulate exp into the group's denominator buffer (DVE)
                ve = nc.vector
                es = state["es"][g]
                if si == 0:
                    if full:
                        ve.tensor_copy(es[:], et[:])
                    else:
                        ve.memset(es[:], 0.0)
                        ve.tensor_add(es[:, :, q0:], es[:, :, q0:],
                                      et[:, :, q0:])
                elif full:
                    ve.tensor_add(es[:], es[:], et[:])
                else:
                    ve.tensor_add(es[:, :, q0:], es[:, :, q0:],
                                  et[:, :, q0:])
                return et

            def emit_av(item):
                g, si, bi, t, kb, q0, et, n, first, last = item
                if first:
                    state["po"][g] = psO.tile([128, QT], F32, tag="po",
                                              name=f"pq{g}")
                po = state["po"][g]
                # col-tiled concurrent pair: hp0 -> psum partitions 0-63
                # (array cols 0-63), hp1 -> 64-127.  The start=True zero
                # marking is per written partition range, so both chains
                # carry start on their first matmul.
                for hp in range(2):
                    nc.tensor.matmul(
                        po[hp * 64:(hp + 1) * 64, q0:QT],
                        v_sb[:, kb, 2 * bi + hp, :],
                        et[:, hp, q0:],
                        start=first, stop=last,
                        skip_group_check=True)
                return n

            def emit_norm(g):
                bi, t, _ = groups[g]
                po = state["po"][g]
                es = state["es"][g]
                ts = slice(t * QT, (t + 1) * QT)
                # reduce the 128 key-partitions of et_sum and broadcast to
                # 64 rows in one col-tiled matmul pair
                bu = psF.tile([128, QT], F32, tag="f", name=f"bu{g}")
                for hp in range(2):
                    nc.tensor.matmul(bu[hp * 64:(hp + 1) * 64, :],
                                     ones64[:], es[:, hp, :],
                                     start=True, stop=True,
                                     skip_group_check=True)
                rec = nrm.tile([128, QT], F32, tag="rec", name=f"rec{g}")
                nc.vector.reciprocal_approx_fast(rec[:], bu[:])
                nc.vector.tensor_mul(ot_sb[bi][:, ts], po[:], rec[:])

            # lagged emission of av (depth 2), norm when group drains
            def retire(min_keep):
                while len(state["pend"]) > min_keep:
                    item = state["pend"].pop(0)
                    drain_key((item[0], item[4]))
                    emit_av(item)
                    g = item[0]
                    state["done_av"][g] = state["done_av"].get(g, 0) + 1
                    if state["done_av"][g] == state["n_in_group"][g]:
                        emit_norm(g)
                        normed.add(g)

            # ---- stage-0 projections for group 0 (m=0) -------------------
            for s in emit_qk_unit("q", 0, 0):
                s()
            for s in emit_qk_unit("k", 0, 0):
                s()

            # ---- filler supply in dependency-safe order ------------------
            fill_plan = {
                0: [("qk", "q", 1, 0), ("qk", "k", 1, 0),
                    ("v", 0), ("v", 1), ("v", 2), ("v", 3)],
                1: [("qk", "q", 0, 1), ("qk", "k", 0, 1)],
                2: [("v", 4), ("v", 5), ("v", 6), ("v", 7),
                    ("qk", "q", 1, 1), ("qk", "k", 1, 1)],
                3: [("qk", "q", 0, 2), ("qk", "k", 0, 2),
                    ("o", 0), ("o", 1), ("o", 2), ("o", 3)],
                4: [("v", 8), ("v", 9), ("v", 10), ("v", 11),
                    ("qk", "q", 1, 2), ("qk", "k", 1, 2),
                    ("o", 4), ("o", 5), ("o", 6), ("o", 7)],
                5: [("qk", "q", 0, 3), ("qk", "k", 0, 3),
                    ("v", 12), ("v", 13), ("v", 14), ("v", 15),
                    ("o", 8), ("o", 9), ("o", 10), ("o", 11)],
                6: [("qk", "q", 1, 3), ("qk", "k", 1, 3),
                    ("o", 12), ("o", 13), ("o", 14), ("o", 15),
                    ("o", 16), ("o", 17), ("o", 18), ("o", 19)],
                7: [("o", 20), ("o", 21), ("o", 22), ("o", 23)],
            }

            def add_fill(g):
                for u in fill_plan.get(g, []):
                    if u[0] == "qk":
                        need = (2 * u[3] + u[2], -1)  # group (bi=m, t=st)
                        filler.extend((need, 512, s)
                                      for s in emit_qk_unit(u[1], u[2], u[3]))
                    elif u[0] == "v":
                        si = u[1]
                        need = (2 * (si // 4), si)    # by av of that kb
                        filler.extend((need, 256, s)
                                      for s in emit_v_unit(si))
                    else:
                        oi = u[1]
                        filler_o.extend((oi // 8, 512, s)
                                        for s in emit_o_unit(oi % 8, oi // 8))

            # ---- main attention loop -------------------------------------
            flat = []
            for g, (bi, t, slots) in enumerate(groups):
                for si, sl in enumerate(slots):
                    flat.append((g, bi, t, si, sl, si == 0,
                                 si == len(slots) - 1))

            def start_group(g):
                bi, t, slots = groups[g]
                add_fill(g)
                drain_key((g, -1))
                state["n_in_group"][g] = len(slots)
                state["es"][g] = nrm.tile([128, 2, QT], BF16, tag="es",
                                          name=f"es{g}")

            def emit_slot_L(j):
                g, bi, t, si, (kb, q0, kind, idx), first, last = flat[j]
                if first:
                    start_group(g)
                return emit_L(bi, t, kb, kind, f"{g}_{si}")

            def emit_slot_fin(j, lg):
                g, bi, t, si, (kb, q0, kind, idx), first, last = flat[j]
                et = emit_fin(g, si, q0, kind, idx, lg, f"{g}_{si}")
                state["pend"].append(
                    (g, si, bi, t, kb, q0, et, QT - q0, first, last))

            # per-slot order: logits pair + triangle + exp, lagged attn@V
            # pair, then filler (full-array) in bursts
            lg0 = emit_slot_L(0)
            emit_slot_fin(0, lg0)
            carry = 0
            for j in range(len(flat)):
                if j + 1 < len(flat):
                    lgn = emit_slot_L(j + 1)
                    emit_slot_fin(j + 1, lgn)
                retire(2)
                g, bi, t, si, (kb, q0, kind, idx), first, last = flat[j]
                n = QT - q0
                # per-slot ACT vs PE cost model (warm clock)
                if q0 == 0 or kind == 2:
                    act_ns = (2 * QT + 352) / 1.2 + 90
                else:
                    act_ns = (2 * n + 352) / 1.2 + 90
                pe_ns = 213 + (120 if kind == 1 else 0) + n / 2.4 + 170
                deficit = int((act_ns - pe_ns) * 2.4) + 150
                if deficit > 0:
                    carry = min(carry + deficit, 8192)
                # pre-drain the upcoming group's projection units so its
                # first logits don't stall behind a 16-matmul burst
                if j + 2 < len(flat) and flat[j + 2][5]:
                    carry = max(carry, 4096)
                if carry >= 1024:
                    carry = max(0, carry - pump(carry))
            retire(0)
            while filler:
                filler.pop(0)[2]()
            while filler_o:
                filler_o.pop(0)[2]()
            # tail: last column block of the output projection; rotate
            # through 6 psum banks (psF pair + both psL tiles' banks), cast
            # on alternating engines, store on all 3 DMA queues
            tailL = [psL.tile([128, 2, QT], F32, tag="lg", name=f"tl{i}")
                     for i in range(2)]
            tail_ps = [None, None, tailL[0][:, 0, :], tailL[0][:, 1, :],
                       tailL[1][:, 0, :], tailL[1][:, 1, :], None, None]
            dqs = (nc.sync, nc.scalar, nc.gpsimd)
            for k, oi in enumerate(range(24, 32)):
                for s in emit_o_unit(oi % 8, oi // 8,
                                     ps_ap=tail_ps[k],
                                     ceng=(nc.vector, nc.scalar)[k % 2],
                                     dq=dqs[k % 3]):
                    s()

    nc.compile()
    return nc


def _plan_from_mask(mask):
    """Classify (t, kb) blocks of the additive mask.

    Returns slot_plan[(t, kb)] = None | (q0, kind, idx), the [128,128]
    triangle tile (NEG-scaled), and full-mask tiles for non-causal blocks.
    """
    m = np.asarray(mask, dtype=np.float32).reshape(S, S)  # [q, k] 1.0=masked
    tri_ref = None
    plan = {}
    full_tiles = []
    full_keys = {}
    for t in range(NQT):
        for kb in range(NKB):
            blk = m[t * QT:(t + 1) * QT, kb * KB:(kb + 1) * KB]  # [q, k]
            if (blk == 1.0).all():
                plan[(t, kb)] = None
                continue
            if not blk.any():
                plan[(t, kb)] = (0, 0, 0)
                continue
            # candidate: leading q0 fully-masked rows + 128-triangle
            fully_masked_rows = (blk == 1.0).all(axis=1)
            q0 = int(np.argmin(fully_masked_rows)) \
                if fully_masked_rows.any() else 0
            if fully_masked_rows[:q0].all() and not \
                    fully_masked_rows[q0:].any():
                qi = np.arange(QT)[:, None]
                ki = np.arange(KB)[None, :]
                expected = (qi < q0 + ki).astype(np.float32)
                if (blk == expected).all() and q0 + KB <= QT:
                    # triangle content in [k, q_rel] layout
                    cand = np.ascontiguousarray(
                        expected[q0:q0 + KB, :].T * NEG).astype(np.float32)
                    if tri_ref is None:
                        tri_ref = cand
                    if (cand == tri_ref).all():
                        plan[(t, kb)] = (q0, 1, 0)
                        continue
            # generic fallback: full [128, 512] additive tile
            tilev = np.ascontiguousarray(blk.T * NEG).astype(np.float32)
            key = tilev.tobytes()
            if key not in full_keys:
                full_keys[key] = len(full_tiles)
                full_tiles.append(tilev)
            # valid q range: first not-fully-masked row
            q0f = int(np.argmin((blk == 1.0).all(axis=1)))
            plan[(t, kb)] = (q0f, 2, full_keys[key])
    if tri_ref is None:
        tri_ref = np.zeros((KB, KB), np.float32)
    return plan, tri_ref, full_tiles


def kernel(query, key_in, value, mask, wq, bq, wk, bk, wv, bv, wo, bo):
    query = np.asarray(query, dtype=np.float32)
    key_in = np.asarray(key_in, dtype=np.float32)
    value = np.asarray(value, dtype=np.float32)
    wq = np.asarray(wq, dtype=np.float32)
    wk = np.asarray(wk, dtype=np.float32)
    wv = np.asarray(wv, dtype=np.float32)
    wo = np.asarray(wo, dtype=np.float32)
    bq = np.asarray(bq, dtype=np.float32)
    bk = np.asarray(bk, dtype=np.float32)
    bv = np.asarray(bv, dtype=np.float32)
    bo = np.asarray(bo, dtype=np.float32)

    plan, tri_tile, full_tiles = _plan_from_mask(mask)
    sig = (tuple(sorted(plan.items())), len(full_tiles))
    if sig not in _cache:
        _cache[sig] = _build(plan, len(full_tiles))
    nc = _cache[sig]

    scale = 1.0 / np.sqrt(np.float32(DEPTH))
    fmask_arr = (np.stack(full_tiles) if full_tiles
                 else np.zeros((1, KB, QT), np.float32))

    bf = ml_dtypes.bfloat16
    xT = {}
    for b in range(B):
        xT[("q", b)] = np.ascontiguousarray(query[b].T).astype(bf)
        xT[("k", b)] = np.ascontiguousarray(key_in[b].T).astype(bf)
        xT[("v", b)] = np.ascontiguousarray(value[b].T).astype(bf)

    in_maps = []
    for c in range(N_CORES):
        b = c // CORES_PER_BATCH
        g = c % CORES_PER_BATCH
        sl = slice(g * DC, (g + 1) * DC)
        in_maps.append({
            "xqT": xT[("q", b)],
            "xkT": xT[("k", b)],
            "xvT": xT[("v", b)],
            "wq": (np.ascontiguousarray(wq[:, sl]) * scale).astype(bf),
            "wk": np.ascontiguousarray(wk[:, sl]).astype(bf),
            "wv": np.ascontiguousarray(wv[:, sl]).astype(bf),
            "wo": np.ascontiguousarray(wo[sl, :]).astype(bf),
            "bq": np.ascontiguousarray((bq[sl] * scale).reshape(2, 128).T),
            "bk": np.ascontiguousarray(bk[sl].reshape(2, 128).T),
            "bv": np.ascontiguousarray(np.broadcast_to(bv[sl], (128, DC))),
            "tri": np.concatenate([tri_tile, tri_tile],
                                  axis=1).astype(ml_dtypes.bfloat16),
            "ident": np.eye(KB, dtype=np.float32).astype(ml_dtypes.bfloat16),
            "fmask": fmask_arr,
        })

    res = run_bass_kernel_spmd(nc, in_maps, list(range(N_CORES)))
    kernel.last_results = res

    out = np.zeros((B, S, D_MODEL), np.float32)
    for b in range(B):
        acc = np.zeros((D_MODEL, S), np.float32)
        for g in range(CORES_PER_BATCH):
            acc += res.results[b * CORES_PER_BATCH + g]["outT"].astype(
                np.float32)
        out[b] = acc.T + bo
    return out


# revision 54
# speedup vs baseline: 1.1787x; 1.1787x over previous
"""Multi-head attention TRN2 kernel (8 NeuronCores), v3.

Sharding: data parallel on batch (B=2, 4 cores each), tensor parallel on
heads (4 of 16 heads per core; wq/wk/wv column-parallel, wo row-parallel).
Each core computes a partial [D, S] transposed output (bf16) for its batch;
the host upcasts, sums the 4 partials per batch, transposes, and adds bo.

v3 vs v2:
- attn@V head pairs run CONCURRENTLY via PE column tiling (M=64 each,
  tile_position (0,0)/(0,64) auto-derived from psum base partitions).  The
  softmax denominator no longer rides the attn@V matmul as a 65th weight
  column; instead the exp tiles are accumulated on the DVE (bf16 et_sum per
  group) and one column-tiled ones-matmul pair reduces+broadcasts the
  denominators at group end.  This also aligns the po psum layout with
  ot_sb rows 0..127, removing the per-group partition-shift DMA.
- exp ops for causally-restricted slots use one 3D-AP ACTIVATE ([128,2,n])
  instead of two, saving the 352-cycle per-op ACT overhead.
- ramp: warmup matmuls bridge the framework preamble to the first DMAs,
  weights+stage0 inputs are split per-chunk across all three DMA queues so
  the first projections trickle-start ~15us earlier; stage1-3 inputs move
  as per-chunk [128,3*512] row-tails (3KB runs) to cut DMA-issue cost.
- tail: the last output-projection units rotate through 6 psum banks, cast
  on alternating DVE/GpSimd, and spread store-DMAs over all 3 queues.
"""

import numpy as np
import ml_dtypes

import concourse.bass as bass
import concourse.mybir as mybir
import concourse.tile as tile
from concourse import bacc, bass_isa
from concourse.bass_utils import run_bass_kernel_spmd

B = 2
S = 2048
D_MODEL = 1024
NUM_HEADS = 16
DEPTH = 64
NEG = -1e9
N_CORES = 8
CORES_PER_BATCH = 4
HEADS_PER_CORE = 4           # 2 head-pairs (bi) x 2 (hp)
DC = HEADS_PER_CORE * DEPTH  # 256
QT = 512                     # query tile
KB = 128                     # key block (one slot per valid (bi, t, kb))
NQT = S // QT                # 4
NKB = S // KB                # 16
KIN = D_MODEL // 128         # 8 contraction chunks

F32 = mybir.dt.float32
BF16 = mybir.dt.bfloat16
EXP = mybir.ActivationFunctionType.Exp

_cache = {}


def _build(slot_plan, n_full_masks):
    """slot_plan[(t, kb)] = None (skip) | (q0, kind, idx)
    kind: 0 plain, 1 tri (leading 128-triangle at q0), 2 full mask tile idx.
    """
    nc = bacc.Bacc("TRN2", target_bir_lowering=False, debug=False,
                   num_devices=N_CORES)

    xqT = nc.dram_tensor("xqT", [D_MODEL, S], BF16, kind="ExternalInput").ap()
    xkT = nc.dram_tensor("xkT", [D_MODEL, S], BF16, kind="ExternalInput").ap()
    xvT = nc.dram_tensor("xvT", [D_MODEL, S], BF16, kind="ExternalInput").ap()
    wq = nc.dram_tensor("wq", [D_MODEL, DC], BF16, kind="ExternalInput").ap()
    wk = nc.dram_tensor("wk", [D_MODEL, DC], BF16, kind="ExternalInput").ap()
    wv = nc.dram_tensor("wv", [D_MODEL, DC], BF16, kind="ExternalInput").ap()
    wo = nc.dram_tensor("wo", [DC, D_MODEL], BF16, kind="ExternalInput").ap()
    bq = nc.dram_tensor("bq", [128, 2], F32, kind="ExternalInput").ap()
    bk = nc.dram_tensor("bk", [128, 2], F32, kind="ExternalInput").ap()
    bv = nc.dram_tensor("bv", [128, DC], F32, kind="ExternalInput").ap()
    tri = nc.dram_tensor("tri", [KB, 2 * KB], BF16,
                         kind="ExternalInput").ap()
    ident = nc.dram_tensor("ident", [KB, KB], BF16,
                           kind="ExternalInput").ap()
    fmask = nc.dram_tensor("fmask", [max(n_full_masks, 1), KB, QT], F32,
                           kind="ExternalInput").ap()
    outT = nc.dram_tensor("outT", [D_MODEL, S], BF16,
                          kind="ExternalOutput").ap()

    with tile.TileContext(nc) as tc:
        import contextlib
        ctx = contextlib.ExitStack()
        with ctx:
            wpool = ctx.enter_context(tc.tile_pool(name="weights", bufs=1))
            xpool = ctx.enter_context(tc.tile_pool(name="xin", bufs=1))
            qkv = ctx.enter_context(tc.tile_pool(name="qkv", bufs=1))
            etp = ctx.enter_context(tc.tile_pool(name="etp", bufs=6))
            nrm = ctx.enter_context(tc.tile_pool(name="nrm", bufs=2))
            ostp = ctx.enter_context(tc.tile_pool(name="ostp", bufs=4))
            psL = ctx.enter_context(
                tc.tile_pool(name="psL", bufs=2, space="PSUM"))
            psO = ctx.enter_context(
                tc.tile_pool(name="psO", bufs=2, space="PSUM"))
            psF = ctx.enter_context(
                tc.tile_pool(name="psF", bufs=2, space="PSUM"))
            # PSUM banks: psL 2x2 + psO 2x1 + psF 2x1 = 8

            # ---- resident weights / constants ----------------------------
            wq_sb = wpool.tile([128, KIN, DC], BF16, tag="wq")
            wk_sb = wpool.tile([128, KIN, DC], BF16, tag="wk")
            wv_sb = wpool.tile([128, KIN, DC], BF16, tag="wv")
            wo_sb = wpool.tile([128, 2, D_MODEL], BF16, tag="wo")
            bq_sb = wpool.tile([128, 2], F32, tag="bq")
            bk_sb = wpool.tile([128, 2], F32, tag="bk")
            bv_sb = wpool.tile([128, DC], F32, tag="bv")
            tri_sb = wpool.tile([KB, 2 * KB], BF16, tag="tri")
            ident_sb = wpool.tile([KB, KB], BF16, tag="ident")
            fm_sb = []
            for i in range(n_full_masks):
                fm = wpool.tile([KB, QT], F32, tag=f"fm{i}", name=f"fm{i}")
                nc.gpsimd.dma_start(fm[:], fmask[i])
                fm_sb.append(fm)

            dummy = wpool.tile([128, QT], BF16, tag="dummy")
            nc.vector.memset(dummy[:], 0.25)
            # [128, 64] of ones: stationary for the denominator
            # reduce+broadcast matmuls (col-tiled pair per group)
            ones64 = wpool.tile([128, 64], BF16, tag="ones64")
            nc.vector.memset(ones64[:], 1.0)

            # persistent activations
            qt_sb = [qkv.tile([128, S], BF16, tag=f"qt{i}", name=f"qt{i}")
                     for i in range(2)]
            kt_sb = [qkv.tile([128, S], BF16, tag=f"kt{i}", name=f"kt{i}")
                     for i in range(2)]
            ot_sb = [qkv.tile([128, S], BF16, tag=f"ot{i}", name=f"ot{i}")
                     for i in range(2)]
            v_sb = qkv.tile([128, NKB, HEADS_PER_CORE, DEPTH], BF16,
                            tag="v")

            # ---- DMA issue ------------------------------------------------
            # warm the exp table ASAP (gated only on dummy memset)
            et_warm = etp.tile([128, 2, QT], BF16, tag="et", name="etwarm")
            nc.scalar.activation(et_warm[:, 0, 0:8], dummy[:, 0:8], EXP)

            # x resident
            xq_sb = xpool.tile([128, KIN, S], BF16, tag="xq")
            xk_sb = xpool.tile([128, KIN, S], BF16, tag="xk")
            xv_sb = xpool.tile([128, KIN, S], BF16, tag="xv")

            # Each HWDGE ring serializes transfers at a ~2us fixed latency
            # apiece, so the ramp-critical loads go as ONE batched transfer
            # per (tensor, stage): q-side on sync, k-side on gpsimd, v-side
            # stage 0 on scalar.  Issue order on a queue is its priority.
            def xst(q, dst, srct, st):
                sl = slice(st * QT, (st + 1) * QT)
                q.dma_start(dst[:, :, sl],
                            srct[:, sl].rearrange("(c p) q -> p c q", p=128))

            # Measured queue speeds: gpsimd (SWDGE) ~210GB/s, scalar ~80,
            # sync ~60.  The bulk flows through gpsimd in need order; the
            # two slow HWDGE rings carry small slices off the critical path.
            def xst0(q, dst, srct, c0, c1):
                q.dma_start(
                    dst[:, c0:c1, 0:QT],
                    srct[c0 * 128:c1 * 128, 0:QT].rearrange(
                        "(c p) q -> p c q", p=128))

            # ramp is HBM-bound across the 8 cores: ship the L0-critical
            # bytes first, per-chunk so compute trickle-starts.
            nc.sync.dma_start(bq_sb[:], bq[:])
            nc.sync.dma_start(
                wq_sb[:], wq.rearrange("(c p) d -> p c d", p=128))
            for ch in range(6):
                nc.sync.dma_start(xq_sb[:, ch, 0:QT],
                                  xqT[ch * 128:(ch + 1) * 128, 0:QT])
            nc.gpsimd.dma_start(bk_sb[:], bk[:])
            nc.gpsimd.dma_start(bv_sb[:], bv[:])
            nc.gpsimd.dma_start(tri_sb[:], tri[:])
            nc.gpsimd.dma_start(ident_sb[:], ident[:])
            nc.gpsimd.dma_start(
                wk_sb[:], wk.rearrange("(c p) d -> p c d", p=128))
            for ch in (6, 7):
                nc.gpsimd.dma_start(xq_sb[:, ch, 0:QT],
                                    xqT[ch * 128:(ch + 1) * 128, 0:QT])
            for ch in range(KIN):
                nc.gpsimd.dma_start(xk_sb[:, ch, 0:QT],
                                    xkT[ch * 128:(ch + 1) * 128, 0:QT])
            # v-side stage 0 spread per-chunk across all three queues so the
            # early attn@V units trickle with the landings
            nc.scalar.dma_start(
                wv_sb[:], wv.rearrange("(c p) d -> p c d", p=128))
            for ch in (6, 7):
                nc.scalar.dma_start(xv_sb[:, ch, 0:QT],
                                    xvT[ch * 128:(ch + 1) * 128, 0:QT])
            for ch in (0, 1, 2):
                nc.sync.dma_start(xv_sb[:, ch, 0:QT],
                                  xvT[ch * 128:(ch + 1) * 128, 0:QT])
            for ch in (3, 4, 5):
                nc.gpsimd.dma_start(xv_sb[:, ch, 0:QT],
                                    xvT[ch * 128:(ch + 1) * 128, 0:QT])
            # stages 1-3 as stage-batched ~1MB transfers, issued in need
            # order: xq/xv on sync, xk on gpsimd
            # xq-st1 gates the first group boundary: ride the fast gpsimd
            # ring (lands ~26us vs ~34us on sync)
            xst(nc.gpsimd, xq_sb, xqT, 1)
            xst(nc.gpsimd, xk_sb, xkT, 1)
            nc.gpsimd.dma_start(
                wo_sb[:], wo.rearrange("(c p) d -> p c d", p=128))
            xst(nc.sync, xv_sb, xvT, 1)
            xst(nc.sync, xq_sb, xqT, 2)
            xst(nc.gpsimd, xk_sb, xkT, 2)
            xst(nc.sync, xv_sb, xvT, 2)
            xst(nc.sync, xq_sb, xqT, 3)
            xst(nc.gpsimd, xk_sb, xkT, 3)
            xst(nc.sync, xv_sb, xvT, 3)

            # ---- PE warmup (HAM) while DMAs land --------------------------
            # a short dummy block, then one matmul per landing xq stage-0
            # chunk.  NOTE: more/denser warmup consistently regresses the
            # whole kernel ~15-20% (power-driven clock throttling) -- keep
            # this minimal.
            for i in range(12):
                pw = psF.tile([128, QT], F32, tag="f", name=f"warm{i}")
                nc.tensor.matmul(pw[:, 0:256], dummy[:, 0:128],
                                 dummy[:, 0:256], start=True, stop=True)
            for ch in range(KIN):
                pw = psF.tile([128, QT], F32, tag="f", name=f"warmx{ch}")
                nc.tensor.matmul(pw[:, 0:256], dummy[:, 0:128],
                                 xq_sb[:, ch, 0:256], start=True, stop=True)

            # ---- filler units --------------------------------------------
            def emit_qk_unit(which, m, st):
                wsb = wq_sb if which == "q" else wk_sb
                bsb = bq_sb if which == "q" else bk_sb
                dst = (qt_sb if which == "q" else kt_sb)[m]
                ps = [None]
                sl = slice(st * QT, (st + 1) * QT)

                xsb = xq_sb if which == "q" else xk_sb

                def mk(ch):
                    def go():
                        if ch == 0:
                            ps[0] = psF.tile([128, QT], F32, tag="f",
                                             name=f"p{which}{m}{st}")
                        nc.tensor.matmul(
                            ps[0][:], wsb[:, ch, m * 128:(m + 1) * 128],
                            xsb[:, ch, sl],
                            start=(ch == 0), stop=(ch == KIN - 1))
                        return 512
                    return go

                steps = [mk(ch) for ch in range(KIN)]

                def fin():
                    nc.vector.tensor_scalar_add(dst[:, sl], ps[0][:],
                                                bsb[:, m:m + 1])
                    return 0
                steps.append(fin)
                return steps

            def emit_v_unit(si):
                ps = [None]

                def mk(ch):
                    def go():
                        if ch == 0:
                            ps[0] = psF.tile([128, QT], F32, tag="f",
                                             name=f"pv{si}")
                        nc.tensor.matmul(
                            ps[0][:, 0:DC],
                            xv_sb[:, ch, si * 128:(si + 1) * 128],
                            wv_sb[:, ch, :],
                            start=(ch == 0), stop=(ch == KIN - 1))
                        return 256
                    return go

                steps = [mk(ch) for ch in range(KIN)]

                def fin():
                    nc.vector.tensor_add(
                        v_sb[:, si, :, :],
                        ps[0][:, 0:DC].rearrange("p (h d) -> p h d",
                                                 h=HEADS_PER_CORE),
                        bv_sb[:].rearrange("p (h d) -> p h d",
                                           h=HEADS_PER_CORE))
                    return 0
                steps.append(fin)
                return steps

            def emit_o_unit(dt, sq, ps_ap=None, ceng=None, dq=None):
                ps = [ps_ap]
                sl = slice(sq * QT, (sq + 1) * QT)

                def mk(bi):
                    def go():
                        if bi == 0 and ps[0] is None:
                            ps[0] = psF.tile([128, QT], F32, tag="f",
                                             name=f"po{dt}{sq}")[:]
                        nc.tensor.matmul(
                            ps[0], wo_sb[:, bi, dt * 128:(dt + 1) * 128],
                            ot_sb[bi][:, sl],
                            start=(bi == 0), stop=(bi == 1))
                        return 512
                    return go

                steps = [mk(bi) for bi in range(2)]

                def fin():
                    ost = ostp.tile([128, QT], BF16, tag="ost",
                                    name=f"os{dt}{sq}")
                    # alternate DVE / ACT for the psum->bf16 cast: DVE is
                    # the busier engine mid-kernel, ACT has idle gaps
                    ce = ceng if ceng is not None else \
                        (nc.vector if dt % 2 == 0 else nc.scalar)
                    if ce is nc.scalar:
                        ce.copy(ost[:], ps[0])
                    else:
                        ce.tensor_copy(ost[:], ps[0])
                    q = dq if dq is not None else \
                        (nc.sync if (dt + sq) % 2 == 0 else nc.gpsimd)
                    q.dma_start(outT[dt * 128:(dt + 1) * 128, sl], ost[:])
                    return 0
                steps.append(fin)
                return steps

            filler = []    # forced entries: (need_key, est, step_fn)
            filler_o = []  # opportunistic o-units: (sq, est, step_fn)
            normed = set()  # groups whose norm has been emitted

            def pump(cycles):
                took = 0
                while filler and took < cycles:
                    took += filler.pop(0)[2]()
                # o-units only once both source groups' norms are emitted
                while filler_o and took < cycles:
                    sq = filler_o[0][0]
                    if 2 * sq in normed and 2 * sq + 1 in normed:
                        took += filler_o.pop(0)[2]()
                    else:
                        break
                return took

            def drain_key(key):
                # scan the whole list: need keys are not globally sorted
                i = 0
                while i < len(filler):
                    if filler[i][0] <= key:
                        filler.pop(i)[2]()
                    else:
                        i += 1

            # ---- attention machinery -------------------------------------
            groups = []
            for t in range(NQT):
                for bi in range(2):
                    slots = []
                    for kb in range(NKB):
                        d = slot_plan.get((t, kb))
                        if d is not None:
                            slots.append((kb,) + d)
                    groups.append((bi, t, slots))

            state = {"pend": [], "po": {}, "es": {}, "done_av": {},
                     "n_in_group": {}}

            def emit_L(bi, t, kb, kind, tag):
                # logits for both head-pairs, row-tiled (K=64 strips at
                # partitions 0/64 -> the two matmuls run concurrently)
                lg = psL.tile([128, 2, QT], F32, tag="lg", name=f"lg{tag}")
                qs = slice(t * QT, (t + 1) * QT)
                for hp in range(2):
                    prow = slice(hp * 64, hp * 64 + 64)
                    nc.tensor.matmul(
                        lg[:, hp, :],
                        kt_sb[bi][prow, kb * KB:(kb + 1) * KB],
                        qt_sb[bi][prow, qs], start=True,
                        stop=(kind != 1))
                return lg

            def emit_fin(g, si, q0, kind, idx, lg, tag):
                if kind == 1:
                    # accumulate the causal triangle on the PE (ident @ tri)
                    for hp in range(2):
                        nc.tensor.matmul(lg[:, hp, q0:q0 + KB], ident_sb[:],
                                         tri_sb[:, 0:KB],
                                         start=False, stop=True)
                elif kind == 2:
                    for hp in range(2):
                        nc.vector.tensor_add(lg[:, hp, :], lg[:, hp, :],
                                             fm_sb[idx][:])
                et = etp.tile([128, 2, QT], BF16, tag="et", name=f"et{tag}")
                full = (q0 == 0 or kind == 2)
                if full:
                    nc.scalar.activation(et[:], lg[:], EXP)
                else:
                    # one 3D-AP op covers both head-pairs' valid columns
                    nc.scalar.activation(et[:, :, q0:], lg[:, :, q0:], EXP)
                # accumulate exp into the group's denominator buffer (DVE)
                ve = nc.vector
                es = state["es"][g]
                if si == 0:
                    if full:
                        ve.tensor_copy(es[:], et[:])
                    else:
                        ve.memset(es[:], 0.0)
                        ve.tensor_add(es[:, :, q0:], es[:, :, q0:],
                                      et[:, :, q0:])
                elif full:
                    ve.tensor_add(es[:], es[:], et[:])
                else:
                    ve.tensor_add(es[:, :, q0:], es[:, :, q0:],
                                  et[:, :, q0:])
                return et

            def emit_av(item):
                g, si, bi, t, kb, q0, et, n, first, last = item
                if first:
                    state["po"][g] = psO.tile([128, QT], F32, tag="po",
                                              name=f"pq{g}")
                po = state["po"][g]
                # col-tiled concurrent pair: hp0 -> psum partitions 0-63
                # (array cols 0-63), hp1 -> 64-127.  The start=True zero
                # marking is per written partition range, so both chains
                # carry start on their first matmul.
                for hp in range(2):
                    nc.tensor.matmul(
                        po[hp * 64:(hp + 1) * 64, q0:QT],
                        v_sb[:, kb, 2 * bi + hp, :],
                        et[:, hp, q0:],
                        start=first, stop=last,
                        skip_group_check=True)
                return n

            def emit_norm(g):
                bi, t, _ = groups[g]
                po = state["po"][g]
                es = state["es"][g]
                ts = slice(t * QT, (t + 1) * QT)
                # reduce the 128 key-partitions of et_sum and broadcast to
                # 64 rows in one col-tiled matmul pair
                bu = psF.tile([128, QT], F32, tag="f", name=f"bu{g}")
                for hp in range(2):
                    nc.tensor.matmul(bu[hp * 64:(hp + 1) * 64, :],
                                     ones64[:], es[:, hp, :],
                                     start=True, stop=True,
                                     skip_group_check=True)
                rec = nrm.tile([128, QT], F32, tag="rec", name=f"rec{g}")
                nc.vector.reciprocal_approx_fast(rec[:], bu[:])
                nc.vector.tensor_mul(ot_sb[bi][:, ts], po[:], rec[:])

            # lagged emission of av (depth 2), norm when group drains
            def retire(min_keep):
                while len(state["pend"]) > min_keep:
                    item = state["pend"].pop(0)
                    drain_key((item[0], item[4]))
                    emit_av(item)
                    g = item[0]
                    state["done_av"][g] = state["done_av"].get(g, 0) + 1
                    if state["done_av"][g] == state["n_in_group"][g]:
                        emit_norm(g)
                        normed.add(g)

            # ---- stage-0 projections for group 0 (m=0) -------------------
            for s in emit_qk_unit("q", 0, 0):
                s()
            for s in emit_qk_unit("k", 0, 0):
                s()

            # ---- filler supply in dependency-safe order ------------------
            fill_plan = {
                0: [("qk", "q", 1, 0), ("qk", "k", 1, 0),
                    ("v", 0), ("v", 1), ("v", 2), ("v", 3)],
                1: [("qk", "q", 0, 1), ("qk", "k", 0, 1)],
                2: [("v", 4), ("v", 5), ("v", 6), ("v", 7),
                    ("qk", "q", 1, 1), ("qk", "k", 1, 1)],
                3: [("qk", "q", 0, 2), ("qk", "k", 0, 2),
                    ("o", 0), ("o", 1), ("o", 2), ("o", 3)],
                4: [("v", 8), ("v", 9), ("v", 10), ("v", 11),
                    ("qk", "q", 1, 2), ("qk", "k", 1, 2),
                    ("o", 4), ("o", 5), ("o", 6), ("o", 7)],
                5: [("qk", "q", 0, 3), ("qk", "k", 0, 3),
                    ("v", 12), ("v", 13), ("v", 14), ("v", 15),
                    ("o", 8), ("o", 9), ("o", 10), ("o", 11)],
                6: [("qk", "q", 1, 3), ("qk", "k", 1, 3),
                    ("o", 12), ("o", 13), ("o", 14), ("o", 15),
                    ("o", 16), ("o", 17), ("o", 18), ("o", 19)],
                7: [("o", 20), ("o", 21), ("o", 22), ("o", 23)],
            }

            def add_fill(g):
                for u in fill_plan.get(g, []):
                    if u[0] == "qk":
                        need = (2 * u[3] + u[2], -1)  # group (bi=m, t=st)
                        filler.extend((need, 512, s)
                                      for s in emit_qk_unit(u[1], u[2], u[3]))
                    elif u[0] == "v":
                        si = u[1]
                        need = (2 * (si // 4), si)    # by av of that kb
                        filler.extend((need, 256, s)
                                      for s in emit_v_unit(si))
                    else:
                        oi = u[1]
                        filler_o.extend((oi // 8, 512, s)
                                        for s in emit_o_unit(oi % 8, oi // 8))

            # ---- main attention loop -------------------------------------
            flat = []
            for g, (bi, t, slots) in enumerate(groups):
                for si, sl in enumerate(slots):
                    flat.append((g, bi, t, si, sl, si == 0,
                                 si == len(slots) - 1))

            def start_group(g):
                bi, t, slots = groups[g]
                add_fill(g)
                drain_key((g, -1))
                state["n_in_group"][g] = len(slots)
                state["es"][g] = nrm.tile([128, 2, QT], BF16, tag="es",
                                          name=f"es{g}")

            def emit_slot_L(j):
                g, bi, t, si, (kb, q0, kind, idx), first, last = flat[j]
                if first:
                    start_group(g)
                return emit_L(bi, t, kb, kind, f"{g}_{si}")

            def emit_slot_fin(j, lg):
                g, bi, t, si, (kb, q0, kind, idx), first, last = flat[j]
                et = emit_fin(g, si, q0, kind, idx, lg, f"{g}_{si}")
                state["pend"].append(
                    (g, si, bi, t, kb, q0, et, QT - q0, first, last))

            # per-slot order: logits pair + triangle + exp, lagged attn@V
            # pair, then filler (full-array) in bursts
            lg0 = emit_slot_L(0)
            emit_slot_fin(0, lg0)
            carry = 0
            for j in range(len(flat)):
                if j + 1 < len(flat):
                    lgn = emit_slot_L(j + 1)
                    emit_slot_fin(j + 1, lgn)
                retire(2)
                g, bi, t, si, (kb, q0, kind, idx), first, last = flat[j]
                n = QT - q0
                # per-slot ACT vs PE cost model (warm clock)
                if q0 == 0 or kind == 2:
                    act_ns = (2 * QT + 352) / 1.2 + 90
                else:
                    act_ns = (2 * n + 352) / 1.2 + 90
                pe_ns = 213 + (120 if kind == 1 else 0) + n / 2.4 + 170
                deficit = int((act_ns - pe_ns) * 2.4) + 150
                if deficit > 0:
                    carry = min(carry + deficit, 8192)
                # pre-drain the upcoming group's projection units so its
                # first logits don't stall behind a 16-matmul burst
                if j + 2 < len(flat) and flat[j + 2][5]:
                    carry = max(carry, 4096)
                if carry >= 1024:
                    carry = max(0, carry - pump(carry))
            retire(0)
            while filler:
                filler.pop(0)[2]()
            while filler_o:
                filler_o.pop(0)[2]()
            # tail: last column block of the output projection; rotate
            # through 6 psum banks (psF pair + both psL tiles' banks), cast
            # on alternating engines, store on all 3 DMA queues
            tailL = [psL.tile([128, 2, QT], F32, tag="lg", name=f"tl{i}")
                     for i in range(2)]
            tail_ps = [None, None, tailL[0][:, 0, :], tailL[0][:, 1, :],
                       tailL[1][:, 0, :], tailL[1][:, 1, :], None, None]
            dqs = (nc.sync, nc.scalar, nc.gpsimd)
            for k, oi in enumerate(range(24, 32)):
                for s in emit_o_unit(oi % 8, oi // 8,
                                     ps_ap=tail_ps[k],
                                     ceng=(nc.vector, nc.scalar)[k % 2],
                                     dq=dqs[k % 3]):
                    s()

    nc.compile()
    return nc


def _plan_from_mask(mask):
    """Classify (t, kb) blocks of the additive mask.

    Returns slot_plan[(t, kb)] = None | (q0, kind, idx), the [128,128]
    triangle tile (NEG-scaled), and full-mask tiles for non-causal blocks.
    """
    m = np.asarray(mask, dtype=np.float32).reshape(S, S)  # [q, k] 1.0=masked
    tri_ref = None
    plan = {}
    full_tiles = []
    full_keys = {}
    for t in range(NQT):
        for kb in range(NKB):
            blk = m[t * QT:(t + 1) * QT, kb * KB:(kb + 1) * KB]  # [q, k]
            if (blk == 1.0).all():
                plan[(t, kb)] = None
                continue
            if not blk.any():
                plan[(t, kb)] = (0, 0, 0)
                continue
            # candidate: leading q0 fully-masked rows + 128-triangle
            fully_masked_rows = (blk == 1.0).all(axis=1)
            q0 = int(np.argmin(fully_masked_rows)) \
                if fully_masked_rows.any() else 0
            if fully_masked_rows[:q0].all() and not \
                    fully_masked_rows[q0:].any():
                qi = np.arange(QT)[:, None]
                ki = np.arange(KB)[None, :]
                expected = (qi < q0 + ki).astype(np.float32)
                if (blk == expected).all() and q0 + KB <= QT:
                    # triangle content in [k, q_rel] layout
                    cand = np.ascontiguousarray(
                        expected[q0:q0 + KB, :].T * NEG).astype(np.float32)
                    if tri_ref is None:
                        tri_ref = cand
                    if (cand == tri_ref).all():
                        plan[(t, kb)] = (q0, 1, 0)
                        continue
            # generic fallback: full [128, 512] additive tile
            tilev = np.ascontiguousarray(blk.T * NEG).astype(np.float32)
            key = tilev.tobytes()
            if key not in full_keys:
                full_keys[key] = len(full_tiles)
                full_tiles.append(tilev)
            # valid q range: first not-fully-masked row
            q0f = int(np.argmin((blk == 1.0).all(axis=1)))
            plan[(t, kb)] = (q0f, 2, full_keys[key])
    if tri_ref is None:
        tri_ref = np.zeros((KB, KB), np.float32)
    return plan, tri_ref, full_tiles


def kernel(query, key_in, value, mask, wq, bq, wk, bk, wv, bv, wo, bo):
    query = np.asarray(query, dtype=np.float32)
    key_in = np.asarray(key_in, dtype=np.float32)
    value = np.asarray(value, dtype=np.float32)
    wq = np.asarray(wq, dtype=np.float32)
    wk = np.asarray(wk, dtype=np.float32)
    wv = np.asarray(wv, dtype=np.float32)
    wo = np.asarray(wo, dtype=np.float32)
    bq = np.asarray(bq, dtype=np.float32)
    bk = np.asarray(bk, dtype=np.float32)
    bv = np.asarray(bv, dtype=np.float32)
    bo = np.asarray(bo, dtype=np.float32)

    plan, tri_tile, full_tiles = _plan_from_mask(mask)
    sig = (tuple(sorted(plan.items())), len(full_tiles))
    if sig not in _cache:
        _cache[sig] = _build(plan, len(full_tiles))
    nc = _cache[sig]

    scale = 1.0 / np.sqrt(np.float32(DEPTH))
    fmask_arr = (np.stack(full_tiles) if full_tiles
                 else np.zeros((1, KB, QT), np.float32))

    bf = ml_dtypes.bfloat16
    xT = {}
    for b in range(B):
        xT[("q", b)] = np.ascontiguousarray(query[b].T).astype(bf)
        xT[("k", b)] = np.ascontiguousarray(key_in[b].T).astype(bf)
        xT[("v", b)] = np.ascontiguousarray(value[b].T).astype(bf)

    in_maps = []
    for c in range(N_CORES):
        b = c // CORES_PER_BATCH
        g = c % CORES_PER_BATCH
        sl = slice(g * DC, (g + 1) * DC)
        in_maps.append({
            "xqT": xT[("q", b)],
            "xkT": xT[("k", b)],
            "xvT": xT[("v", b)],
            "wq": (np.ascontiguousarray(wq[:, sl]) * scale).astype(bf),
            "wk": np.ascontiguousarray(wk[:, sl]).astype(bf),
            "wv": np.ascontiguousarray(wv[:, sl]).astype(bf),
            "wo": np.ascontiguousarray(wo[sl, :]).astype(bf),
            "bq": np.ascontiguousarray((bq[sl] * scale).reshape(2, 128).T),
            "bk": np.ascontiguousarray(bk[sl].reshape(2, 128).T),
            "bv": np.ascontiguousarray(np.broadcast_to(bv[sl], (128, DC))),
            "tri": np.concatenate([tri_tile, tri_tile],
                                  axis=1).astype(ml_dtypes.bfloat16),
            "ident": np.eye(KB, dtype=np.float32).astype(ml_dtypes.bfloat16),
            "fmask": fmask_arr,
        })

    res = run_bass_kernel_spmd(nc, in_maps, list(range(N_CORES)))
    kernel.last_results = res

    out = np.zeros((B, S, D_MODEL), np.float32)
    for b in range(B):
        acc = np.zeros((D_MODEL, S), np.float32)
        for g in range(CORES_PER_BATCH):
            acc += res.results[b * CORES_PER_BATCH + g]["outT"].astype(
                np.float32)
        out[b] = acc.T + bo
    return out
